# revision 49
# baseline (speedup 1.0000x reference)
"""Trainium2 Bass kernel for nn_DotAttention (B=8 data-parallel over 8 cores).

Per core (one batch element), bf16 with one fp8 DoubleRow stage:
  x.T/m.T   : fp32 PE transposes, PSUM->SBUF copy casts to bf16; emitted
              lazily per 4-tile group, interleaved with the attention loop
  xp/mp     : relu(W.T @ {x,m}.T + b), bf16, T layout [96, 2048]
  S.T       : mp.T(:,jtile) @ xp.T, bf16 (K=96, 128-col stationary -> FWL)
  e8        : exp(S.T*scale + maskbias) -> fp8e4 directly from ACT
  U[jx,151] : fp8 DoubleRow, NATURAL layout: e8[jm,2,jx128]^T @ [m|1];
              contracts 256 keys/instr; softmax denominator lands in col 150
  normalize : reciprocal_approx_fast + per-partition tensor_scalar -> bf16
  gate      : natural layout; stationary res.T chunks, moving Wg [g,300];
              bias via ones-row in res.T paired with a bg row in Wg;
              x-tail/U-tail/ones merged into one 65-row contraction
  out       : sigmoid -> SBUF, elementwise * [x_nat | U16n], natural DMA out
Pipeline: exps of pair t precede next-pair scores which precede U(t); h1's
normalize/U.T-transposes interleave; tail runs h1 gates first, then h0
transposes + gates. All within 8 PSUM banks (sp 2x2, U 3, preamble 1).
"""

import contextlib
import math

import numpy as np

import concourse.bass as bass
import concourse.mybir as mybir
import concourse.tile as tile
from concourse import bacc
from concourse.bass_utils import run_bass_kernel_spmd
from concourse.masks import make_identity

F32 = mybir.dt.float32
F16 = mybir.dt.bfloat16
F8 = mybir.dt.float8e4
I32 = mybir.dt.int32
DR = mybir.MatmulPerfMode.DoubleRow

B = 8
JX = 2048
JM = 2048
D = 150
H = 96
G = 300
NJT = 16          # jm tiles of 128
NCH = 16          # jx chunks of 128
HALF = 1024
NSUB = HALF // 512
SCALE = 1.0 / math.sqrt(float(H))
NEG_BIG = 1.0e30
WSCALE = 8.0      # Wi/Wm pre-scale for fp8 range
GSCALE = 16.0     # Wg pre-scale


def _body(tc, x_d, m_d, mask_d, wi_d, bi_d, wm_d, bm_d, wg_d, bg_d, o_d):
    nc = tc.nc
    Relu = mybir.ActivationFunctionType.Relu
    Exp = mybir.ActivationFunctionType.Exp
    Sigmoid = mybir.ActivationFunctionType.Sigmoid
    MUL = mybir.AluOpType.mult
    SUB = mybir.AluOpType.subtract

    with contextlib.ExitStack() as ctx:
        const = ctx.enter_context(tc.tile_pool(name="const", bufs=1))
        work = ctx.enter_context(tc.tile_pool(name="work", bufs=2))
        epool = ctx.enter_context(tc.tile_pool(name="epool", bufs=3))
        psb = ctx.enter_context(tc.tile_pool(name="psb", bufs=2, space="PSUM"))
        pu = ctx.enter_context(tc.tile_pool(name="pu", bufs=1, space="PSUM"))

        ident16 = const.tile([128, 128], F16)
        make_identity(nc, ident16)
        ident32s = const.tile([NJT, NJT], F32)
        make_identity(nc, ident32s)
        ident32 = const.tile([128, 128], F32)
        make_identity(nc, ident32)

        # preload the exp table set (covers exp/relu/copy) during DMA wait
        dummy = const.tile([1, 1], F32)
        nc.scalar.activation(out=dummy, in_=ident16[0:1, 0:1], func=Exp, scale=1.0)

        # warm-up matmuls: ramp the PE p-state during the input DMA wait so
        # the first transposes run at full clock
        jp = psb.tile([128, 128], F32, tag="big", name="junk")
        for _ in range(40):
            nc.tensor.matmul(
                jp, ident16, ident16, start=True, stop=True,
                skip_group_check=True)
        nc.vector.tensor_copy(out=dummy, in_=jp[0:1, 0:1])

        # ---- weights (small, first on the sync queue) --------------------
        wstage = const.tile([128, 2 * H], F32)
        nc.sync.dma_start(out=wstage[:, 0:H], in_=wi_d[0:128, :])
        nc.sync.dma_start(out=wstage[:, H : 2 * H], in_=wm_d[0:128, :])
        wstage2 = const.tile([D - 128, 2 * H], F32)
        nc.sync.dma_start(out=wstage2[:, 0:H], in_=wi_d[128:D, :])
        nc.sync.dma_start(out=wstage2[:, H : 2 * H], in_=wm_d[128:D, :])
        bi_sb = const.tile([H, 1], F32)
        nc.sync.dma_start(out=bi_sb, in_=bi_d.rearrange("(n one) -> n one", one=1))
        bm_sb = const.tile([H, 1], F32)
        nc.sync.dma_start(out=bm_sb, in_=bm_d.rearrange("(n one) -> n one", one=1))
        wi16a = const.tile([128, H], F16)
        nc.vector.tensor_copy(out=wi16a, in_=wstage[:, 0:H])
        wi16b = const.tile([D - 128, H], F16)
        nc.vector.tensor_copy(out=wi16b, in_=wstage2[:, 0:H])
        wm16a = const.tile([128, H], F16)
        nc.vector.tensor_copy(out=wm16a, in_=wstage[:, H : 2 * H])
        wm16b = const.tile([D - 128, H], F16)
        nc.vector.tensor_copy(out=wm16b, in_=wstage2[:, H : 2 * H])

        # ---- mask -> additive exp bias [128, NJT] ------------------------
        mask_sb = const.tile([NJT, 128], I32)
        nc.sync.dma_start(out=mask_sb, in_=mask_d.rearrange("(n p) -> n p", p=128))
        maskf = const.tile([NJT, 128], F32)
        nc.vector.tensor_copy(out=maskf, in_=mask_sb)
        nc.vector.tensor_scalar(
            out=maskf, in0=maskf, scalar1=1.0, scalar2=NEG_BIG,
            op0=SUB, op1=MUL)
        mb_ps = psb.tile([128, NJT], F32, tag="gp", name="mbps", bufs=1)
        nc.tensor.transpose(mb_ps, maskf, ident32s)
        maskbias = const.tile([128, NJT], F32)
        nc.vector.tensor_copy(out=maskbias, in_=mb_ps)

        # ---- inputs: stream in, cast, transpose --------------------------
        x_nat = const.tile([128, NCH, D], F32)
        m_nat = const.tile([128, NJT, D], F32)
        x_re = x_d.rearrange("(n p) d -> p n d", p=128)
        m_re = m_d.rearrange("(n p) d -> p n d", p=128)
        # fp8 natural m for the U matmuls (cols 0..149 = m, 150 = 1.0)
        mt8 = const.tile([128, NJT, 176], F8)
        nc.gpsimd.memset(mt8[:, :, D:176], 0.0)
        nc.gpsimd.memset(mt8[:, :, 150:151], 1.0)

        xT16a = const.tile([128, JX], F16)
        mT16a = const.tile([128, JM], F16)
        mT16b = const.tile([D - 128, JM], F16)
        # merged tail: x.T tail rows 0..21, U.T tail rows 32..53, ones row 64
        rtail = const.tile([65, JX], F16)
        nc.vector.memset(rtail, 0.0)
        nc.vector.memset(rtail[64:65, :], 1.0)
        xT16b = rtail

        for g in range(4):
            gs4 = slice(g * 4, (g + 1) * 4)
            nc.sync.dma_start(out=m_nat[:, gs4, :], in_=m_re[:, gs4, :])
            nc.scalar.dma_start(out=x_nat[:, gs4, :], in_=x_re[:, gs4, :])
            nc.gpsimd.tensor_copy(out=mt8[:, gs4, 0:D], in_=m_nat[:, gs4, :])

        def transpose_group(src32, dstA, dstB, g, pool_m=True):
            # one 4-chunk group (512 cols) as two fp32 2-chunk pieces; the
            # PSUM->SBUF copy performs the f32->bf16 cast
            for p2 in range(2):
                if pool_m:
                    pT = psb.tile([128, 2, 256], F32, tag="gp", name="pT",
                                  bufs=1)
                else:
                    pT = pu.tile([128, 2, 256], F32, tag="U", name="pTx")
                for i in range(2):
                    c = g * 4 + p2 * 2 + i
                    nc.tensor.transpose(
                        pT[:, i, 0:128], src32[:, c, 0:128], ident32)
                    nc.tensor.transpose(
                        pT[0 : D - 128, i, 128:256], src32[:, c, 128:D],
                        ident32)
                gcols = slice(g * 512 + p2 * 256, g * 512 + (p2 + 1) * 256)
                nc.vector.tensor_copy(out=dstA[:, gcols], in_=pT[:, :, 0:128])
                nc.vector.tensor_copy(
                    out=dstB[0 : D - 128, gcols],
                    in_=pT[0 : D - 128, :, 128:256])

        # Wg/bg late (needed only at gate time): moving operands, fp16.
        # 4 g-chunks; the last one carries an extra ones-row (g row 22)
        # paired with bg as the matching Wg row -> bias via matmul.
        wg16a = const.tile([128, G], F16, tag="wg16a")
        wg16c = const.tile([128, G], F16, tag="wg16c")
        wgtail = const.tile([65, G], F16, tag="wgtail")
        nc.gpsimd.memset(wgtail, 0.0)
        for sl, (g0, g1), w, r0 in ((0, (0, 128), wg16a, 0),
                                    (1, (128, 150), wgtail, 0),
                                    (2, (150, 278), wg16c, 0),
                                    (3, (278, 300), wgtail, 32)):
            wst = const.tile([g1 - g0, G], F32, tag=f"wgst_{sl}", name=f"wgst{sl}")
            nc.sync.dma_start(out=wst, in_=wg_d[g0:g1, :])
            nc.gpsimd.tensor_copy(out=w[r0 : r0 + g1 - g0, :], in_=wst)
        bgst = const.tile([1, G], F32, tag="bgst")
        nc.sync.dma_start(out=bgst, in_=bg_d.rearrange("(one n) -> one n", one=1))
        nc.gpsimd.tensor_copy(out=wgtail[64:65, :], in_=bgst)

        # ---- projections -> xpT16/mpT16, emitted per 512-col sub ---------
        xpT16 = const.tile([H, JX], F16)
        mpT16 = const.tile([H, JM], F16)

        def proj_sub(wa, wb, srcA, srcB, dst, sub, pool_m=True):
            ss = slice(sub * 512, (sub + 1) * 512)
            if pool_m:
                pp = psb.tile([128, 512], F32, tag="gp", name="pp", bufs=1)
            else:
                pp = pu.tile([128, 512], F32, tag="U", name="ppx")
            nc.tensor.matmul(
                pp[0:H, :], wa, srcA[:, ss],
                start=True, stop=False, skip_group_check=True)
            nc.tensor.matmul(
                pp[0:H, :], wb, srcB[0 : D - 128, ss],
                start=False, stop=True, skip_group_check=True)
            b_sb = bm_sb if dst is mpT16 else bi_sb
            nc.scalar.activation(
                out=dst[:, ss], in_=pp[0:H, :], func=Relu, bias=b_sb, scale=1.0)

        # lazy emission state: how many m/x groups transposed + projected
        state = {"mg": 0, "xg": 0}
        uT16a = const.tile([128, JX], F16)

        def ut_group(g, pre=False):
            # transpose U16n chunks 2g, 2g+1 into uT16a / rtail rows 32..53;
            # pre=True uses the preamble's gp bank (idle during attention)
            if pre:
                pA = psb.tile([128, 2, 256], F16, tag="gp", name="pUAg",
                              bufs=1)
            else:
                pA = psb.tile([128, 2, 256], F16, tag="big", name="pUA")
            for i in range(2):
                c = g * 2 + i
                nc.tensor.transpose(
                    pA[:, i, 0:128], U16n[:, c, 0:128], ident16)
                nc.tensor.transpose(
                    pA[0 : D - 128, i, 128:256], U16n[:, c, 128:D], ident16)
            gcols = slice(g * 256, (g + 1) * 256)
            nc.vector.tensor_copy(out=uT16a[:, gcols], in_=pA[:, :, 0:128])
            nc.vector.tensor_copy(
                out=rtail[32 : 32 + D - 128, gcols],
                in_=pA[0 : D - 128, :, 128:256])

        def need_m(jtiles):
            # ensure mpT16 covers j-tiles < jtiles (each group = 4 tiles)
            while state["mg"] * 4 < jtiles:
                g = state["mg"]
                transpose_group(m_nat, mT16a, mT16b, g)
                proj_sub(wm16a, wm16b, mT16a, mT16b, mpT16, g)
                state["mg"] = g + 1

        def need_x(chunks):
            while state["xg"] * 4 < chunks:
                g = state["xg"]
                pm = g >= 2  # pre-attention groups may use the idle U bank
                transpose_group(x_nat, xT16a, rtail, g, pool_m=pm)
                proj_sub(wi16a, wi16b, xT16a, rtail, xpT16, g, pool_m=pm)
                state["xg"] = g + 1

        # ---- attention per jx half ---------------------------------------
        U16n = const.tile([128, NCH, 160], F16)
        nc.vector.memset(U16n[:, :, 150:160], 0.0)
        rcp_all = const.tile([128, NCH], F32)
        o_re = o_d.rearrange("(n p) k -> p n k", p=128)
        gate16 = const.tile([128, NCH, G], F16)

        def gate_chunk(c):
            cs = slice(c * 128, (c + 1) * 128)
            gp = psb.tile([128, 304], F32, tag="big", name="gp")
            for gi, (lhs, w) in enumerate((
                (xT16a[:, cs], wg16a), (uT16a[:, cs], wg16c),
                (rtail[:, cs], wgtail))):
                nc.tensor.matmul(
                    gp[:, 0:G], lhs, w,
                    start=(gi == 0), stop=(gi == 2), skip_group_check=True)
            nc.scalar.activation(
                out=gate16[:, c, :], in_=gp[:, 0:G], func=Sigmoid, scale=1.0)

        def out_pair(cp):
            c2 = slice(cp * 2, cp * 2 + 2)
            onat = work.tile([128, 2, G], F32, tag="onat", bufs=4)
            eng = nc.vector if cp % 2 == 0 or cp == 3 else nc.gpsimd
            eng.tensor_tensor(
                out=onat[:, :, 0:D], in0=gate16[:, c2, 0:D],
                in1=x_nat[:, c2, :], op=MUL)
            eng.tensor_tensor(
                out=onat[:, :, D:G], in0=gate16[:, c2, D:G],
                in1=U16n[:, c2, 0:D], op=MUL)
            nc.sync.dma_start(out=o_re[:, c2, :], in_=onat)
        def emit_scores_h(h, j):
            need_m(min(j + 3, NJT) if h == 0 else j + 1)
            sp = psb.tile([128, HALF], F32, tag="big", name="sp")
            for sx in range(NSUB):
                ss = slice(h * HALF + sx * 512, h * HALF + (sx + 1) * 512)
                nc.tensor.matmul(
                    sp[:, sx * 512 : (sx + 1) * 512],
                    mpT16[:, j * 128 : (j + 1) * 128], xpT16[:, ss],
                    start=True, stop=True, skip_group_check=True)
            return sp

        need_x(8)
        sps = [emit_scores_h(0, 0), emit_scores_h(0, 1)]
        for h in range(2):
            hs = slice(h * HALF, (h + 1) * HALF)
            Up = pu.tile([128, 8, 152], F32, tag="U", name="Up")
            e_cur = epool.tile([128, 2, HALF], F8, tag="e8", name="e8")
            for t in range(NJT // 2):
                for s in range(2):
                    j = 2 * t + s
                    nc.scalar.activation(
                        out=e_cur[:, s, :], in_=sps[s], func=Exp,
                        bias=maskbias[:, j : j + 1], scale=SCALE)
                if t < NJT // 2 - 1:
                    sps = [emit_scores_h(h, 2 * t + 2),
                           emit_scores_h(h, 2 * t + 3)]
                elif h == 0:
                    # cross-boundary prefetch: h1's first scores run during
                    # h0's last exps so the exp stream never stalls
                    sps = [emit_scores_h(1, 0), emit_scores_h(1, 1)]
                if h == 0 and t in (1, 3):
                    need_x(8 + 4 * ((t + 1) // 2))
                if h == 1 and t % 2 == 1:
                    ut_group(t // 2, pre=True)
                for c in range(8):
                    nc.tensor.matmul(
                        Up[:, c, 0:151],
                        e_cur[:, :, c * 128 : (c + 1) * 128],
                        mt8[:, 2 * t : 2 * t + 2, 0:151],
                        start=(t == 0), stop=(t == NJT // 2 - 1),
                        perf_mode=DR, skip_group_check=True)

                if t < NJT // 2 - 1:
                    e_cur = epool.tile([128, 2, HALF], F8, tag="e8", name="e8")

            # normalize (DVE only; h0's overlaps h1's attention; h1's is
            # interleaved with its U.T transposes to shorten the tail)
            hc = slice(h * 8, h * 8 + 8)
            den = work.tile([128, 8], F32, tag="den")
            nc.vector.tensor_copy(out=den, in_=Up[:, :, 150])
            nc.vector.reciprocal_approx_fast(out=rcp_all[:, hc], in_=den)

            def norm_chunk(c):
                nc.vector.tensor_scalar(
                    out=U16n[:, c, 0:D], in0=Up[:, c - h * 8, 0:D],
                    scalar1=rcp_all[:, c : c + 1],
                    scalar2=None, op0=MUL)

            if h == 0:
                for c in range(8):
                    norm_chunk(c)
            else:
                nc.scalar.activation(
                    out=dummy, in_=ident16[0:1, 0:1],
                    func=Sigmoid, scale=1.0)
                for g in range(4, 8):
                    norm_chunk(2 * g)
                    norm_chunk(2 * g + 1)
                    ut_group(g, pre=True)
                    gate_chunk(2 * g)
                    gate_chunk(2 * g + 1)

        # ---- tail: h0 gates keep PE/ACT streaming, then all outputs ------
        for c in range(8):
            gate_chunk(c)
        for cp in range(4, 8):
            out_pair(cp)
        for cp in range(4):
            out_pair(cp)


_NC_CACHE = None


def _build_nc():
    global _NC_CACHE
    if _NC_CACHE is not None:
        return _NC_CACHE
    nc = bacc.Bacc(None, target_bir_lowering=False, debug=False)
    x_d = nc.dram_tensor("x", [JX, D], F32, kind="ExternalInput")
    m_d = nc.dram_tensor("m", [JM, D], F32, kind="ExternalInput")
    mask_d = nc.dram_tensor("mask", [JM], I32, kind="ExternalInput")
    wi_d = nc.dram_tensor("Wi", [D, H], F32, kind="ExternalInput")
    bi_d = nc.dram_tensor("bi", [H], F32, kind="ExternalInput")
    wm_d = nc.dram_tensor("Wm", [D, H], F32, kind="ExternalInput")
    bm_d = nc.dram_tensor("bm", [H], F32, kind="ExternalInput")
    wg_d = nc.dram_tensor("Wg", [G, G], F32, kind="ExternalInput")
    bg_d = nc.dram_tensor("bg", [G], F32, kind="ExternalInput")
    o_d = nc.dram_tensor("out", [JX, G], F32, kind="ExternalOutput")
    with tile.TileContext(nc) as tc:
        _body(tc, x_d, m_d, mask_d, wi_d, bi_d, wm_d, bm_d, wg_d, bg_d, o_d)
    nc.finalize()
    _NC_CACHE = nc
    return nc


def _in_maps(inputs, memory, mask, Wi, bi, Wm, bm, Wg, bg):
    maps = []
    for b in range(B):
        maps.append(
            {
                "x": np.ascontiguousarray(inputs[b], dtype=np.float32),
                "m": np.ascontiguousarray(memory[b], dtype=np.float32),
                "mask": np.ascontiguousarray(mask[b], dtype=np.int32),
                "Wi": np.ascontiguousarray(Wi, dtype=np.float32),
                "bi": np.ascontiguousarray(bi, dtype=np.float32),
                "Wm": np.ascontiguousarray(Wm, dtype=np.float32),
                "bm": np.ascontiguousarray(bm, dtype=np.float32),
                "Wg": np.ascontiguousarray(Wg, dtype=np.float32),
                "bg": np.ascontiguousarray(bg, dtype=np.float32),
            }
        )
    return maps


def run_spmd(inputs, memory, mask, Wi, bi, Wm, bm, Wg, bg, **spmd_kwargs):
    """Run the kernel across 8 cores; returns the BassKernelResults."""
    nc = _build_nc()
    maps = _in_maps(
        np.asarray(inputs), np.asarray(memory), np.asarray(mask),
        np.asarray(Wi), np.asarray(bi), np.asarray(Wm), np.asarray(bm),
        np.asarray(Wg), np.asarray(bg),
    )
    return run_bass_kernel_spmd(nc, maps, list(range(B)), **spmd_kwargs)


def kernel(inputs, memory, mask, Wi, bi, Wm, bm, Wg, bg):
    res = run_spmd(inputs, memory, mask, Wi, bi, Wm, bm, Wg, bg)
    out = np.stack([res.results[b]["out"] for b in range(B)], axis=0)
    return out.astype(np.float32)


# revision 51
# speedup vs baseline: 1.1008x; 1.1008x over previous
"""Trainium2 Bass kernel for nn_DotAttention (B=8 data-parallel over 8 cores).

Per core (one batch element), bf16 with one fp8 DoubleRow stage:
  x.T/m.T   : fp32 PE transposes, PSUM->SBUF copy casts to bf16; emitted
              lazily per 4-tile group, interleaved with the attention loop
  xp/mp     : relu(W.T @ {x,m}.T + b), bf16, T layout [96, 2048]
  S.T       : mp.T(:,jtile) @ xp.T, bf16 (K=96, 128-col stationary -> FWL)
  e8        : exp(S.T*scale + maskbias) -> fp8e4 directly from ACT
  U[jx,151] : fp8 DoubleRow, NATURAL layout: e8[jm,2,jx128]^T @ [m|1];
              contracts 256 keys/instr; softmax denominator lands in col 150
  normalize : reciprocal_approx_fast + per-partition tensor_scalar -> bf16
  gate      : natural layout; stationary res.T chunks, moving Wg [g,300];
              bias via ones-row in res.T paired with a bg row in Wg;
              x-tail/U-tail/ones merged into one 65-row contraction
  out       : sigmoid -> SBUF, elementwise * [x_nat | U16n], natural DMA out
Pipeline: exps of pair t precede next-pair scores which precede U(t); h1's
normalize/U.T-transposes interleave; tail runs h1 gates first, then h0
transposes + gates. All within 8 PSUM banks (sp 2x2, U 3, preamble 1).
"""

import contextlib
import math

import numpy as np

import concourse.bass as bass
import concourse.mybir as mybir
import concourse.tile as tile
from concourse import bacc
from concourse.bass_utils import run_bass_kernel_spmd
from concourse.masks import make_identity

F32 = mybir.dt.float32
F16 = mybir.dt.bfloat16
F8 = mybir.dt.float8e4
I32 = mybir.dt.int32
DR = mybir.MatmulPerfMode.DoubleRow

B = 8
JX = 2048
JM = 2048
D = 150
H = 96
G = 300
NJT = 16          # jm tiles of 128
NCH = 16          # jx chunks of 128
HALF = 1024
NSUB = HALF // 512
SCALE = 1.0 / math.sqrt(float(H))
NEG_BIG = 1.0e30
WSCALE = 8.0      # Wi/Wm pre-scale for fp8 range
GSCALE = 16.0     # Wg pre-scale


def _body(tc, x_d, m_d, mask_d, wi_d, bi_d, wm_d, bm_d, wg_d, bg_d, o_d):
    nc = tc.nc
    Relu = mybir.ActivationFunctionType.Relu
    Exp = mybir.ActivationFunctionType.Exp
    Sigmoid = mybir.ActivationFunctionType.Sigmoid
    MUL = mybir.AluOpType.mult
    SUB = mybir.AluOpType.subtract

    with contextlib.ExitStack() as ctx:
        const = ctx.enter_context(tc.tile_pool(name="const", bufs=1))
        work = ctx.enter_context(tc.tile_pool(name="work", bufs=2))
        epool = ctx.enter_context(tc.tile_pool(name="epool", bufs=3))
        psb = ctx.enter_context(tc.tile_pool(name="psb", bufs=2, space="PSUM"))
        pu = ctx.enter_context(tc.tile_pool(name="pu", bufs=1, space="PSUM"))

        ident16 = const.tile([128, 128], F16)
        make_identity(nc, ident16)
        ident32s = const.tile([NJT, NJT], F32)
        make_identity(nc, ident32s)
        ident32 = const.tile([128, 128], F32)
        make_identity(nc, ident32)

        # preload the exp table set (covers exp/relu/copy) during DMA wait
        dummy = const.tile([1, 1], F32)
        nc.scalar.activation(out=dummy, in_=ident16[0:1, 0:1], func=Exp, scale=1.0)

        # warm-up matmuls: ramp the PE p-state during the input DMA wait so
        # the first transposes run at full clock
        jp = psb.tile([128, 128], F32, tag="big", name="junk")
        for _ in range(40):
            nc.tensor.matmul(
                jp, ident16, ident16, start=True, stop=True,
                skip_group_check=True)
        nc.vector.tensor_copy(out=dummy, in_=jp[0:1, 0:1])

        # ---- weights (small, first on the sync queue) --------------------
        wstage = const.tile([128, 2 * H], F32)
        nc.sync.dma_start(out=wstage[:, 0:H], in_=wi_d[0:128, :])
        nc.sync.dma_start(out=wstage[:, H : 2 * H], in_=wm_d[0:128, :])
        wstage2 = const.tile([D - 128, 2 * H], F32)
        nc.sync.dma_start(out=wstage2[:, 0:H], in_=wi_d[128:D, :])
        nc.sync.dma_start(out=wstage2[:, H : 2 * H], in_=wm_d[128:D, :])
        bi_sb = const.tile([H, 1], F32)
        nc.sync.dma_start(out=bi_sb, in_=bi_d.rearrange("(n one) -> n one", one=1))
        bm_sb = const.tile([H, 1], F32)
        nc.sync.dma_start(out=bm_sb, in_=bm_d.rearrange("(n one) -> n one", one=1))
        wi16a = const.tile([128, H], F16)
        nc.vector.tensor_copy(out=wi16a, in_=wstage[:, 0:H])
        wi16b = const.tile([D - 128, H], F16)
        nc.vector.tensor_copy(out=wi16b, in_=wstage2[:, 0:H])
        wm16a = const.tile([128, H], F16)
        nc.vector.tensor_copy(out=wm16a, in_=wstage[:, H : 2 * H])
        wm16b = const.tile([D - 128, H], F16)
        nc.vector.tensor_copy(out=wm16b, in_=wstage2[:, H : 2 * H])

        # ---- mask -> additive exp bias [128, NJT] ------------------------
        mask_sb = const.tile([NJT, 128], I32)
        nc.sync.dma_start(out=mask_sb, in_=mask_d.rearrange("(n p) -> n p", p=128))
        maskf = const.tile([NJT, 128], F32)
        nc.vector.tensor_copy(out=maskf, in_=mask_sb)
        nc.vector.tensor_scalar(
            out=maskf, in0=maskf, scalar1=1.0, scalar2=NEG_BIG,
            op0=SUB, op1=MUL)
        mb_ps = psb.tile([128, NJT], F32, tag="gp", name="mbps", bufs=1)
        nc.tensor.transpose(mb_ps, maskf, ident32s)
        maskbias = const.tile([128, NJT], F32)
        nc.vector.tensor_copy(out=maskbias, in_=mb_ps)

        # ---- inputs: stream in, cast, transpose --------------------------
        x_nat = const.tile([128, NCH, D], F32)
        m_nat = const.tile([128, NJT, D], F32)
        x_re = x_d.rearrange("(n p) d -> p n d", p=128)
        m_re = m_d.rearrange("(n p) d -> p n d", p=128)
        # fp8 natural m for the U matmuls (cols 0..149 = m, 150 = 1.0)
        mt8 = const.tile([128, NJT, 176], F8)
        nc.gpsimd.memset(mt8[:, :, D:176], 0.0)
        nc.gpsimd.memset(mt8[:, :, 150:151], 1.0)

        xT16a = const.tile([128, JX], F16)
        mT16a = const.tile([128, JM], F16)
        mT16b = const.tile([D - 128, JM], F16)
        # merged tail: x.T tail rows 0..21, U.T tail rows 32..53, ones row 64
        rtail = const.tile([65, JX], F16)
        nc.vector.memset(rtail, 0.0)
        nc.vector.memset(rtail[64:65, :], 1.0)
        xT16b = rtail

        for g in range(4):
            gs4 = slice(g * 4, (g + 1) * 4)
            nc.sync.dma_start(out=m_nat[:, gs4, :], in_=m_re[:, gs4, :])
            nc.scalar.dma_start(out=x_nat[:, gs4, :], in_=x_re[:, gs4, :])
            nc.gpsimd.tensor_copy(out=mt8[:, gs4, 0:D], in_=m_nat[:, gs4, :])

        def transpose_group(src32, dstA, dstB, g, pool_m=True):
            # one 4-chunk group (512 cols) as two fp32 2-chunk pieces; the
            # PSUM->SBUF copy performs the f32->bf16 cast
            for p2 in range(2):
                if pool_m:
                    pT = psb.tile([128, 2, 256], F32, tag="gp", name="pT",
                                  bufs=1)
                else:
                    pT = pu.tile([128, 2, 256], F32, tag="U", name="pTx")
                for i in range(2):
                    c = g * 4 + p2 * 2 + i
                    nc.tensor.transpose(
                        pT[:, i, 0:128], src32[:, c, 0:128], ident32)
                    nc.tensor.transpose(
                        pT[0 : D - 128, i, 128:256], src32[:, c, 128:D],
                        ident32)
                gcols = slice(g * 512 + p2 * 256, g * 512 + (p2 + 1) * 256)
                nc.vector.tensor_copy(out=dstA[:, gcols], in_=pT[:, :, 0:128])
                nc.vector.tensor_copy(
                    out=dstB[0 : D - 128, gcols],
                    in_=pT[0 : D - 128, :, 128:256])

        # Wg/bg late (needed only at gate time): moving operands, fp16.
        # 4 g-chunks; the last one carries an extra ones-row (g row 22)
        # paired with bg as the matching Wg row -> bias via matmul.
        wg16a = const.tile([128, G], F16, tag="wg16a")
        wg16c = const.tile([128, G], F16, tag="wg16c")
        wgtail = const.tile([65, G], F16, tag="wgtail")
        nc.gpsimd.memset(wgtail, 0.0)
        for sl, (g0, g1), w, r0 in ((0, (0, 128), wg16a, 0),
                                    (1, (128, 150), wgtail, 0),
                                    (2, (150, 278), wg16c, 0),
                                    (3, (278, 300), wgtail, 32)):
            wst = const.tile([g1 - g0, G], F32, tag=f"wgst_{sl}", name=f"wgst{sl}")
            nc.sync.dma_start(out=wst, in_=wg_d[g0:g1, :])
            nc.gpsimd.tensor_copy(out=w[r0 : r0 + g1 - g0, :], in_=wst)
        bgst = const.tile([1, G], F32, tag="bgst")
        nc.sync.dma_start(out=bgst, in_=bg_d.rearrange("(one n) -> one n", one=1))
        nc.gpsimd.tensor_copy(out=wgtail[64:65, :], in_=bgst)

        # ---- projections -> xpT16/mpT16, emitted per 512-col sub ---------
        xpT16 = const.tile([H, JX], F16)
        mpT16 = const.tile([H, JM], F16)

        def proj_sub(wa, wb, srcA, srcB, dst, sub, pool_m=True):
            ss = slice(sub * 512, (sub + 1) * 512)
            if pool_m:
                pp = psb.tile([128, 512], F32, tag="gp", name="pp", bufs=1)
            else:
                pp = pu.tile([128, 512], F32, tag="U", name="ppx")
            nc.tensor.matmul(
                pp[0:H, :], wa, srcA[:, ss],
                start=True, stop=False, skip_group_check=True)
            nc.tensor.matmul(
                pp[0:H, :], wb, srcB[0 : D - 128, ss],
                start=False, stop=True, skip_group_check=True)
            b_sb = bm_sb if dst is mpT16 else bi_sb
            nc.scalar.activation(
                out=dst[:, ss], in_=pp[0:H, :], func=Relu, bias=b_sb, scale=1.0)

        # lazy emission state: how many m/x groups transposed + projected
        state = {"mg": 0, "xg": 0}
        uT16a = const.tile([128, JX], F16)

        def ut_group(g, pre=False):
            # transpose U16n chunks 2g, 2g+1 into uT16a / rtail rows 32..53;
            # pre=True uses the preamble's gp bank (idle during attention)
            if pre:
                pA = psb.tile([128, 2, 256], F16, tag="gp", name="pUAg",
                              bufs=1)
            else:
                pA = psb.tile([128, 2, 256], F16, tag="big", name="pUA")
            for i in range(2):
                c = g * 2 + i
                nc.tensor.transpose(
                    pA[:, i, 0:128], U16n[:, c, 0:128], ident16)
                nc.tensor.transpose(
                    pA[0 : D - 128, i, 128:256], U16n[:, c, 128:D], ident16)
            gcols = slice(g * 256, (g + 1) * 256)
            nc.vector.tensor_copy(out=uT16a[:, gcols], in_=pA[:, :, 0:128])
            nc.vector.tensor_copy(
                out=rtail[32 : 32 + D - 128, gcols],
                in_=pA[0 : D - 128, :, 128:256])

        def need_m(jtiles):
            # ensure mpT16 covers j-tiles < jtiles (each group = 4 tiles)
            while state["mg"] * 4 < jtiles:
                g = state["mg"]
                transpose_group(m_nat, mT16a, mT16b, g)
                proj_sub(wm16a, wm16b, mT16a, mT16b, mpT16, g)
                state["mg"] = g + 1

        def need_x(chunks):
            while state["xg"] * 4 < chunks:
                g = state["xg"]
                pm = g >= 2  # pre-attention groups may use the idle U bank
                transpose_group(x_nat, xT16a, rtail, g, pool_m=pm)
                proj_sub(wi16a, wi16b, xT16a, rtail, xpT16, g, pool_m=pm)
                state["xg"] = g + 1

        # ---- attention per jx half ---------------------------------------
        U16n = const.tile([128, NCH, 160], F16)
        nc.vector.memset(U16n[:, :, 150:160], 0.0)
        rcp_all = const.tile([128, NCH], F32)
        o_re = o_d.rearrange("(n p) k -> p n k", p=128)
        gate16 = const.tile([128, NCH, G], F16)

        def gate_chunk(c):
            cs = slice(c * 128, (c + 1) * 128)
            gp = psb.tile([128, 304], F32, tag="big", name="gp")
            for gi, (lhs, w) in enumerate((
                (xT16a[:, cs], wg16a), (uT16a[:, cs], wg16c),
                (rtail[:, cs], wgtail))):
                nc.tensor.matmul(
                    gp[:, 0:G], lhs, w,
                    start=(gi == 0), stop=(gi == 2), skip_group_check=True)
            nc.scalar.activation(
                out=gate16[:, c, :], in_=gp[:, 0:G], func=Sigmoid, scale=1.0)

        def out_pair(cp):
            c2 = slice(cp * 2, cp * 2 + 2)
            onat = work.tile([128, 2, G], F32, tag="onat", bufs=4)
            eng = nc.vector if cp % 2 == 0 or cp == 3 else nc.gpsimd
            eng.tensor_tensor(
                out=onat[:, :, 0:D], in0=gate16[:, c2, 0:D],
                in1=x_nat[:, c2, :], op=MUL)
            eng.tensor_tensor(
                out=onat[:, :, D:G], in0=gate16[:, c2, D:G],
                in1=U16n[:, c2, 0:D], op=MUL)
            nc.sync.dma_start(out=o_re[:, c2, :], in_=onat)
        def emit_scores_h(h, j):
            need_m(min(j + 3, NJT) if h == 0 else j + 1)
            sp = psb.tile([128, HALF], F32, tag="big", name="sp")
            for sx in range(NSUB):
                ss = slice(h * HALF + sx * 512, h * HALF + (sx + 1) * 512)
                nc.tensor.matmul(
                    sp[:, sx * 512 : (sx + 1) * 512],
                    mpT16[:, j * 128 : (j + 1) * 128], xpT16[:, ss],
                    start=True, stop=True, skip_group_check=True)
            return sp

        need_x(8)
        sps = [emit_scores_h(0, 0), emit_scores_h(0, 1)]
        for h in range(2):
            hs = slice(h * HALF, (h + 1) * HALF)
            Up = pu.tile([128, 8, 152], F32, tag="U", name="Up")
            e_cur = epool.tile([128, 2, HALF], F8, tag="e8", name="e8")
            for t in range(NJT // 2):
                for s in range(2):
                    j = 2 * t + s
                    nc.scalar.activation(
                        out=e_cur[:, s, :], in_=sps[s], func=Exp,
                        bias=maskbias[:, j : j + 1], scale=SCALE)
                if t < NJT // 2 - 1:
                    sps = [emit_scores_h(h, 2 * t + 2),
                           emit_scores_h(h, 2 * t + 3)]
                elif h == 0:
                    # cross-boundary prefetch: h1's first scores run during
                    # h0's last exps so the exp stream never stalls
                    sps = [emit_scores_h(1, 0), emit_scores_h(1, 1)]
                if h == 0 and t in (2, 4):
                    need_x(8 + 4 * (t // 2))
                if h == 1 and t % 2 == 1:
                    ut_group(t // 2, pre=True)
                for c in range(8):
                    nc.tensor.matmul(
                        Up[:, c, 0:151],
                        e_cur[:, :, c * 128 : (c + 1) * 128],
                        mt8[:, 2 * t : 2 * t + 2, 0:151],
                        start=(t == 0), stop=(t == NJT // 2 - 1),
                        perf_mode=DR, skip_group_check=True)

                if t < NJT // 2 - 1:
                    e_cur = epool.tile([128, 2, HALF], F8, tag="e8", name="e8")

            # normalize (DVE only; h0's overlaps h1's attention; h1's is
            # interleaved with its U.T transposes to shorten the tail)
            hc = slice(h * 8, h * 8 + 8)
            den = work.tile([128, 8], F32, tag="den")
            nc.vector.tensor_copy(out=den, in_=Up[:, :, 150])
            nc.vector.reciprocal_approx_fast(out=rcp_all[:, hc], in_=den)

            def norm_chunk(c):
                nc.vector.tensor_scalar(
                    out=U16n[:, c, 0:D], in0=Up[:, c - h * 8, 0:D],
                    scalar1=rcp_all[:, c : c + 1],
                    scalar2=None, op0=MUL)

            if h == 0:
                for c in range(8):
                    norm_chunk(c)
            else:
                nc.scalar.activation(
                    out=dummy, in_=ident16[0:1, 0:1],
                    func=Sigmoid, scale=1.0)
                for g in range(4, 8):
                    norm_chunk(2 * g)
                    norm_chunk(2 * g + 1)
                    ut_group(g, pre=True)
                    gate_chunk(2 * g)
                    gate_chunk(2 * g + 1)

        # ---- tail: h0 gates keep PE/ACT streaming, then all outputs ------
        for c in range(8):
            gate_chunk(c)
        for cp in range(4, 8):
            out_pair(cp)
        for cp in range(4):
            out_pair(cp)


_NC_CACHE = None


def _build_nc():
    global _NC_CACHE
    if _NC_CACHE is not None:
        return _NC_CACHE
    nc = bacc.Bacc(None, target_bir_lowering=False, debug=False)
    x_d = nc.dram_tensor("x", [JX, D], F32, kind="ExternalInput")
    m_d = nc.dram_tensor("m", [JM, D], F32, kind="ExternalInput")
    mask_d = nc.dram_tensor("mask", [JM], I32, kind="ExternalInput")
    wi_d = nc.dram_tensor("Wi", [D, H], F32, kind="ExternalInput")
    bi_d = nc.dram_tensor("bi", [H], F32, kind="ExternalInput")
    wm_d = nc.dram_tensor("Wm", [D, H], F32, kind="ExternalInput")
    bm_d = nc.dram_tensor("bm", [H], F32, kind="ExternalInput")
    wg_d = nc.dram_tensor("Wg", [G, G], F32, kind="ExternalInput")
    bg_d = nc.dram_tensor("bg", [G], F32, kind="ExternalInput")
    o_d = nc.dram_tensor("out", [JX, G], F32, kind="ExternalOutput")
    with tile.TileContext(nc) as tc:
        _body(tc, x_d, m_d, mask_d, wi_d, bi_d, wm_d, bm_d, wg_d, bg_d, o_d)
    nc.finalize()
    _NC_CACHE = nc
    return nc


def _in_maps(inputs, memory, mask, Wi, bi, Wm, bm, Wg, bg):
    maps = []
    for b in range(B):
        maps.append(
            {
                "x": np.ascontiguousarray(inputs[b], dtype=np.float32),
                "m": np.ascontiguousarray(memory[b], dtype=np.float32),
                "mask": np.ascontiguousarray(mask[b], dtype=np.int32),
                "Wi": np.ascontiguousarray(Wi, dtype=np.float32),
                "bi": np.ascontiguousarray(bi, dtype=np.float32),
                "Wm": np.ascontiguousarray(Wm, dtype=np.float32),
                "bm": np.ascontiguousarray(bm, dtype=np.float32),
                "Wg": np.ascontiguousarray(Wg, dtype=np.float32),
                "bg": np.ascontiguousarray(bg, dtype=np.float32),
            }
        )
    return maps


def run_spmd(inputs, memory, mask, Wi, bi, Wm, bm, Wg, bg, **spmd_kwargs):
    """Run the kernel across 8 cores; returns the BassKernelResults."""
    nc = _build_nc()
    maps = _in_maps(
        np.asarray(inputs), np.asarray(memory), np.asarray(mask),
        np.asarray(Wi), np.asarray(bi), np.asarray(Wm), np.asarray(bm),
        np.asarray(Wg), np.asarray(bg),
    )
    return run_bass_kernel_spmd(nc, maps, list(range(B)), **spmd_kwargs)


def kernel(inputs, memory, mask, Wi, bi, Wm, bm, Wg, bg):
    res = run_spmd(inputs, memory, mask, Wi, bi, Wm, bm, Wg, bg)
    out = np.stack([res.results[b]["out"] for b in range(B)], axis=0)
    return out.astype(np.float32)
